# revision 1
# baseline (speedup 1.0000x reference)
"""nn_Encoder_627065225609: window-attention encoder on 8 NeuronCores.

Strategy: the geodesic window partition (gather by argsort(window_ids)) and
its inverse are the same permutation in every layer, and every other op is
per-token or per-window. So: permute once on the host, run the entire
4-layer encoder in the sorted (window-contiguous) domain — where the work is
fully data-parallel over the 1280 (B*NW) windows — shard 160 windows per
core via pmap (no collectives needed), and apply the inverse permutation
once at the end. fp32 throughout, matmuls at Precision.HIGHEST.
"""
import numpy as np
import jax
import jax.numpy as jnp

B, N, C = 4, 20480, 128
H, HD = 8, 16
L = 4
NW, WS = 320, 64
SCALE = HD ** -0.5
EPS = 1e-5
M = 8  # cores

_PREC = jax.lax.Precision.HIGHEST


def _ln(x, g, b):
    mu = jnp.mean(x, axis=-1, keepdims=True)
    var = jnp.mean(jnp.square(x - mu), axis=-1, keepdims=True)
    return (x - mu) * jax.lax.rsqrt(var + EPS) * g + b


def _encoder_shard(y, params):
    """y: [T, C] tokens of this shard's windows, window-contiguous."""
    T = y.shape[0]
    W = T // WS

    def step(x, p):
        g1, be1, Wqkv, bqkv, rb, Wp, bp, g2, be2, W1, b1, W2, b2 = p
        shortcut = x
        win = x.reshape(W, WS, C)
        h = _ln(win, g1, be1)
        qkv = (h @ Wqkv + bqkv).reshape(W, WS, 3, H, HD)
        # keep q/k/v in native [W, WS, H, HD] layout; let dot_general carry
        # the (w, h) batch dims — avoids XLA's 5D DVE-transpose kernels
        q, k, v = qkv[:, :, 0], qkv[:, :, 1], qkv[:, :, 2]
        attn = jnp.einsum('wqhd,wkhd->whqk', q, k, precision=_PREC) * SCALE
        attn = attn + rb[None]
        attn = jax.nn.softmax(attn, axis=-1)
        out = jnp.einsum('whqk,wkhd->wqhd', attn, v, precision=_PREC)
        out = out.reshape(W * WS, C)
        out = jnp.dot(out, Wp, precision=_PREC) + bp
        x = shortcut + out
        h2 = _ln(x, g2, be2)
        hid = jax.nn.gelu(jnp.dot(h2, W1, precision=_PREC) + b1)
        x = x + jnp.dot(hid, W2, precision=_PREC) + b2
        return x, None

    x, _ = jax.lax.scan(step, y, params)
    return x


_CACHE = {}


def kernel(x, g1, be1, Wqkv, bqkv, rel_bias, Wproj, bproj, g2, be2,
           W1, b1, W2, b2, window_ids):
    x = np.asarray(x)
    in_dtype = x.dtype
    wid = np.asarray(window_ids)
    sort_idx = np.argsort(wid, kind='stable')

    # Host gather into sorted (window-contiguous) domain, then shard over
    # the B*NW window axis: 8 cores x 160 windows x 64 tokens.
    y = np.ascontiguousarray(x[:, sort_idx, :], dtype=np.float32)
    shards = y.reshape(M, (B * NW // M) * WS, C)

    devs = jax.devices()[:M]
    if 'fn' not in _CACHE:
        _CACHE['fn'] = jax.pmap(_encoder_shard, in_axes=(0, None),
                                devices=devs)
    fn = _CACHE['fn']

    pkey = id(Wqkv)
    if _CACHE.get('pkey') != pkey:
        _CACHE['params'] = tuple(
            jnp.asarray(np.asarray(a), dtype=jnp.float32)
            for a in (g1, be1, Wqkv, bqkv, rel_bias, Wproj, bproj,
                      g2, be2, W1, b1, W2, b2))
        _CACHE['pkey'] = pkey
    params = _CACHE['params']

    dsh = jax.device_put_sharded([shards[i] for i in range(M)], devs)
    out = fn(dsh, params)
    out_sh = np.asarray(jax.device_get(out))

    # Unshard + inverse permutation.
    y_out = out_sh.reshape(B, N, C)
    x_out = np.empty_like(y_out)
    x_out[:, sort_idx, :] = y_out
    return x_out.astype(in_dtype, copy=False)

